# revision 14
# baseline (speedup 1.0000x reference)
"""GCN layer kernel for Trainium2 (Bass/Tile), data-parallel over batch.

Reference computation (per batch element):
    deg = A.sum(-1); d = deg ** -0.5
    t   = X @ W.T + b
    out = relu(diag(d) @ A @ diag(d) @ t)

Per-core mapping (8 cores, one batch element each). Host-side staging is
layout/dtype only (transposes + bf16 rounding, same rounding the device
matmul path would apply); all model arithmetic (degree, normalization,
matmuls, bias, relu) runs on device:
  - A is staged twice in bf16: AT (transposed, the matmul stationary) and
    AN (natural, for the on-device degree row-sums). Streaming over the
    contraction index k, AT row-tile k provides the stationary chunks for
    ALL 16 output tiles, so each step runs a uniform batch of 16 products
    (k, mu) — no triangular schedule and no on-device transposes.
  - deg row-sums on DVE (bf16 2x rate) from AN tiles; d = sqrt(1/deg).
  - t = X @ W.T + b in bf16 from host-staged XT/WT; the bias is folded in
    as a K=1 ones x b product that initializes each accumulation group.
    y[k] = d[k] * t[k] rounded to bf16 by ACT.
  - All 16 output chains accumulate in PSUM f32 simultaneously, packed
    2-per-bank across all 8 banks (half-bank sharing: the bank's first
    matmul uses start=True, which marks the whole 2KB zero-region
    pending-zero; the partner chain's first matmul uses start=False and
    overwrites its still-pending half; the bank's last matmul carries
    stop=True). Banks 6/7 first serve the mm1 staging, then host chains
    12..15.
  - Drain: relu(d * psum) on ACT, stores batched 4 row-tiles per
    gpsimd (SWDGE) dma_start.
"""

from contextlib import ExitStack

import numpy as np
import ml_dtypes

import concourse.bacc as bacc
import concourse.mybir as mybir
import concourse.tile as tile
from concourse.bass_utils import run_bass_kernel_spmd

B = 8
N = 2048
F = 256
P = 128
NT = N // P  # 16 row tiles
FT = F // P  # 2 feature chunks
NP_ = NT // 2  # 8 load pairs
F32 = mybir.dt.float32
BF16 = mybir.dt.bfloat16
COPY = mybir.ActivationFunctionType.Copy
RELU = mybir.ActivationFunctionType.Relu
PF_PAIRS = 3  # pairs of (AT, AN) tiles prefetched ahead
STORE_BATCH = 2
WARMUP_MMS = 40  # dummy matmuls in the idle head to trip the PE HAM un-throttle


def _emit(ctx: ExitStack, tc: tile.TileContext, AT, AN, XT, WTB, BIASB, OUT):
    nc = tc.nc

    const = ctx.enter_context(tc.tile_pool(name="const", bufs=1))
    at_stage = ctx.enter_context(tc.tile_pool(name="at_stage", bufs=5))
    an_stage = ctx.enter_context(tc.tile_pool(name="an_stage", bufs=5))
    scr = ctx.enter_context(tc.tile_pool(name="scr", bufs=3))
    outstage = ctx.enter_context(tc.tile_pool(name="outstage", bufs=2))
    psum_acc = ctx.enter_context(tc.tile_pool(name="psum_acc", bufs=6, space="PSUM"))
    psum_tr = ctx.enter_context(tc.tile_pool(name="psum_tr", bufs=2, space="PSUM"))

    # ---- head DMA: first A pair leads, small bf16 operands slot behind ----
    at_tiles = {}
    an_tiles = {}

    def emit_load_pair(pr):
        an_tiles[pr] = an_stage.tile([P, 2 * N], BF16, tag="an", name=f"an_{pr}")
        nc.sync.dma_start(
            out=an_tiles[pr][:, :].rearrange("p (t n) -> p t n", t=2),
            in_=AN.rearrange("(t p) n -> p t n", p=P)[:, 2 * pr : 2 * pr + 2, :],
        )
        at_tiles[pr] = at_stage.tile([P, 2 * N], BF16, tag="at", name=f"at_{pr}")
        nc.sync.dma_start(
            out=at_tiles[pr][:, :].rearrange("p (t n) -> p t n", t=2),
            in_=AT.rearrange("(t p) n -> p t n", p=P)[:, 2 * pr : 2 * pr + 2, :],
        )

    emit_load_pair(0)
    # small constants ride the second HWDGE queue (ACT) alongside the A stream
    wt_sb = const.tile([P, FT * F], BF16, tag="wt")
    nc.scalar.dma_start(
        out=wt_sb[:, :].rearrange("p (c f) -> p c f", c=FT),
        in_=WTB.rearrange("(c p) f -> p c f", p=P),
    )
    b_bf = const.tile([1, F], BF16, tag="bbf")
    nc.scalar.dma_start(out=b_bf[:, :], in_=BIASB[:, :])
    xt_sb = const.tile([P, FT * N], BF16, tag="xt")
    nc.scalar.dma_start(
        out=xt_sb[:, :].rearrange("p (c n) -> p c n", c=FT),
        in_=XT.rearrange("(c p) n -> p c n", p=P),
    )
    for pr in range(1, PF_PAIRS):
        emit_load_pair(pr)

    # PE warmup: dummy matmuls with no data deps fill the idle head so the
    # HAM activity window un-throttles the PE clock before the real work
    from concourse.masks import make_identity

    ident = const.tile([P, P], BF16, tag="ident")
    make_identity(nc, ident[:, :])
    warm = psum_tr.tile([P, P], F32, tag="tr", name="warm")
    for _ in range(WARMUP_MMS):
        nc.tensor.matmul(warm[:, :], ident[:, :], ident[:, :], start=True, stop=True)

    ones_bf = const.tile([1, P], BF16, tag="ones")
    nc.vector.memset(ones_bf[:, :], 1.0)

    deg = const.tile([P, NT], F32, tag="deg")
    rec = const.tile([P, NT], F32, tag="rec")
    dinv = const.tile([P, NT], F32, tag="dinv")
    t_big = const.tile([P, NT * F], F32, tag="t")
    y_big = const.tile([P, NT * F], BF16, tag="y")

    # ---- mm1: t[j] = X @ W.T + b, bf16, through the tr-bank rotation ----
    for j in range(NT):
        t_psum = psum_tr.tile([P, F], F32, tag="tr", name=f"t_psum_{j}")
        nc.tensor.matmul(t_psum[:, :], ones_bf[:, :], b_bf[:, :], start=True, stop=False)
        for phi in range(FT):
            nc.tensor.matmul(
                t_psum[:, :],
                xt_sb[:, phi * N + j * P : phi * N + (j + 1) * P],
                wt_sb[:, phi * F : (phi + 1) * F],
                start=False,
                stop=(phi == FT - 1),
            )
        nc.scalar.copy(t_big[:, j * F : (j + 1) * F], t_psum[:, :])

    # ---- all 16 accumulation chains, 2 per bank ----
    acc_banks = [
        psum_acc.tile([P, 2 * F], F32, tag="acc", name=f"accbank_{b_}")
        for b_ in range(6)
    ]
    cbank_a = psum_tr.tile([P, 2 * F], F32, tag="tr", name="cbank_a")  # chains 12,13
    cbank_b = psum_tr.tile([P, 2 * F], F32, tag="tr", name="cbank_b")  # chains 14,15

    def acc_region(mu):
        half = (mu % 2) * F
        if mu < 12:
            return acc_banks[mu // 2][:, half : half + F]
        return (cbank_a if mu < 14 else cbank_b)[:, half : half + F]

    ostiles = {}

    def emit_drain(mu):
        bi = mu // STORE_BATCH
        if bi not in ostiles:
            ostiles[bi] = outstage.tile(
                [P, STORE_BATCH * F], F32, tag="os", name=f"os_{bi}"
            )
        j = mu % STORE_BATCH
        dst = ostiles[bi][:, j * F : (j + 1) * F]
        if mu % 2 == 0:
            # relu(d * psum) on ACT
            nc.scalar.activation(
                dst, acc_region(mu), RELU, scale=dinv[:, mu : mu + 1]
            )
        else:
            # same on DVE: (psum * d) max 0
            nc.vector.tensor_scalar(
                out=dst,
                in0=acc_region(mu),
                scalar1=dinv[:, mu : mu + 1],
                scalar2=0.0,
                op0=mybir.AluOpType.mult,
                op1=mybir.AluOpType.max,
            )
        if j == STORE_BATCH - 1:
            lo = bi * STORE_BATCH
            q = nc.sync if (bi % 2 == 0) else nc.gpsimd
            q.dma_start(
                out=OUT.rearrange("(m p) f -> p m f", p=P)[:, lo : lo + STORE_BATCH, :],
                in_=ostiles[bi][:, :].rearrange("p (m f) -> p m f", m=STORE_BATCH),
            )

    # ---- stream over the contraction index k ----
    for pr in range(NP_):
        if pr + PF_PAIRS < NP_:
            emit_load_pair(pr + PF_PAIRS)
        at_pair = at_tiles.pop(pr)
        an_pair = an_tiles.pop(pr)
        for h in range(2):
            k = 2 * pr + h
            # degree row-sums, split across DVE (even k) and ACT (odd k)
            sc = scr.tile([P, N], BF16, tag="sc", name=f"sc_{k}")
            if k % 2 == 0:
                nc.vector.tensor_scalar(
                    out=sc[:, :],
                    in0=an_pair[:, h * N : (h + 1) * N],
                    scalar1=0.0,
                    scalar2=None,
                    op0=mybir.AluOpType.add,
                    op1=mybir.AluOpType.add,
                    accum_out=deg[:, k : k + 1],
                )
            else:
                nc.scalar.activation(
                    sc[:, :],
                    an_pair[:, h * N : (h + 1) * N],
                    COPY,
                    accum_out=deg[:, k : k + 1],
                )
            nc.vector.reciprocal(rec[:, k : k + 1], deg[:, k : k + 1])
            nc.scalar.sqrt(dinv[:, k : k + 1], rec[:, k : k + 1])
            nc.scalar.activation(
                y_big[:, k * F : (k + 1) * F],
                t_big[:, k * F : (k + 1) * F],
                COPY,
                scale=dinv[:, k : k + 1],
            )
            # one uniform batch of products: every output tile consumes y[k]
            for mu in range(NT):
                nc.tensor.matmul(
                    acc_region(mu),
                    at_pair[:, h * N + mu * P : h * N + (mu + 1) * P],
                    y_big[:, k * F : (k + 1) * F],
                    start=(k == 0 and mu % 2 == 0),
                    stop=(k == NT - 1 and mu % 2 == 1),
                )

    # ---- tail: relu(d * acc) and batched stores ----
    for mu in range(NT):
        emit_drain(mu)


_cached_nc = None


def _build():
    nc = bacc.Bacc("TRN2", target_bir_lowering=False, debug=False)
    AT = nc.dram_tensor("at", [N, N], BF16, kind="ExternalInput").ap()
    AN = nc.dram_tensor("an", [N, N], BF16, kind="ExternalInput").ap()
    XT = nc.dram_tensor("xt", [F, N], BF16, kind="ExternalInput").ap()
    WTB = nc.dram_tensor("wtb", [F, F], BF16, kind="ExternalInput").ap()
    BIASB = nc.dram_tensor("biasb", [1, F], BF16, kind="ExternalInput").ap()
    OUT = nc.dram_tensor("out", [N, F], F32, kind="ExternalOutput").ap()
    with tile.TileContext(nc) as tc:
        with ExitStack() as ctx:
            _emit(ctx, tc, AT, AN, XT, WTB, BIASB, OUT)
    nc.compile()
    return nc


def get_nc():
    global _cached_nc
    if _cached_nc is None:
        _cached_nc = _build()
    return _cached_nc


def make_in_maps(node_features, adj_matrix, W, b):
    bf16 = ml_dtypes.bfloat16
    node_features = np.asarray(node_features, dtype=np.float32)
    adj_matrix = np.asarray(adj_matrix, dtype=np.float32)
    an = adj_matrix.astype(bf16)  # [B, N, N] natural
    at = np.ascontiguousarray(an.transpose(0, 2, 1))  # [B, N, N] transposed
    xt = np.ascontiguousarray(
        node_features.astype(bf16).transpose(0, 2, 1)
    )  # [B, F, N]
    wtb = np.ascontiguousarray(np.asarray(W, dtype=np.float32).T.astype(bf16))
    biasb = np.ascontiguousarray(
        np.asarray(b, dtype=np.float32).reshape(1, F).astype(bf16)
    )
    return [
        {
            "at": np.ascontiguousarray(at[c]),
            "an": np.ascontiguousarray(an[c]),
            "xt": xt[c],
            "wtb": wtb,
            "biasb": biasb,
        }
        for c in range(B)
    ]


def kernel(node_features, adj_matrix, W, b):
    nc = get_nc()
    in_maps = make_in_maps(node_features, adj_matrix, W, b)
    res = run_bass_kernel_spmd(nc, in_maps, core_ids=list(range(B)))
    return np.stack([r["out"] for r in res.results], axis=0)


# revision 18
# speedup vs baseline: 1.2336x; 1.2336x over previous
"""GCN layer kernel for Trainium2 (Bass/Tile), data-parallel over batch.

Reference computation (per batch element):
    deg = A.sum(-1); d = deg ** -0.5
    t   = X @ W.T + b
    out = relu(diag(d) @ A @ diag(d) @ t)

Per-core mapping (8 cores, one batch element each). Host-side staging is
layout/dtype only (transposes + bf16 rounding, same rounding the device
matmul path would apply); all model arithmetic (degree, normalization,
matmuls, bias, relu) runs on device:
  - A is staged twice in bf16: AT (transposed, the matmul stationary) and
    AN (natural, for the on-device degree row-sums). Streaming over the
    contraction index k, AT row-tile k provides the stationary chunks for
    ALL 16 output tiles, so each step runs a uniform batch of 16 products
    (k, mu) — no triangular schedule and no on-device transposes.
  - deg row-sums on DVE (bf16 2x rate) from AN tiles; d = sqrt(1/deg).
  - t = X @ W.T + b in bf16 from host-staged XT/WT; the bias is folded in
    as a K=1 ones x b product that initializes each accumulation group.
    y[k] = d[k] * t[k] rounded to bf16 by ACT.
  - All 16 output chains accumulate in PSUM f32 simultaneously, packed
    2-per-bank across all 8 banks (half-bank sharing: the bank's first
    matmul uses start=True, which marks the whole 2KB zero-region
    pending-zero; the partner chain's first matmul uses start=False and
    overwrites its still-pending half; the bank's last matmul carries
    stop=True). Banks 6/7 first serve the mm1 staging, then host chains
    12..15.
  - Drain: relu(d * psum) on ACT, stores batched 4 row-tiles per
    gpsimd (SWDGE) dma_start.
"""

from contextlib import ExitStack

import numpy as np
import ml_dtypes

import concourse.bacc as bacc
import concourse.mybir as mybir
import concourse.tile as tile
from concourse.bass_utils import run_bass_kernel_spmd

B = 8
N = 2048
F = 256
P = 128
NT = N // P  # 16 row tiles
FT = F // P  # 2 feature chunks
NP_ = NT // 2  # 8 load pairs
F32 = mybir.dt.float32
BF16 = mybir.dt.bfloat16
COPY = mybir.ActivationFunctionType.Copy
RELU = mybir.ActivationFunctionType.Relu
PF_PAIRS = 3  # pairs of (AT, AN) tiles prefetched ahead
STORE_BATCH = 4
WARMUP_MMS = 60  # dummy matmuls in the idle head to trip the PE HAM un-throttle
LAG_TR = 2  # steps by which chains 12..15 lag (their banks host mm1 first)


def _emit(ctx: ExitStack, tc: tile.TileContext, AT, AN, XT, WTB, BIASB, OUT):
    nc = tc.nc

    const = ctx.enter_context(tc.tile_pool(name="const", bufs=1))
    at_stage = ctx.enter_context(tc.tile_pool(name="at_stage", bufs=5))
    an_stage = ctx.enter_context(tc.tile_pool(name="an_stage", bufs=5))
    scr = ctx.enter_context(tc.tile_pool(name="scr", bufs=3))
    outstage = ctx.enter_context(tc.tile_pool(name="outstage", bufs=4))
    psum_acc = ctx.enter_context(tc.tile_pool(name="psum_acc", bufs=6, space="PSUM"))
    psum_tr = ctx.enter_context(tc.tile_pool(name="psum_tr", bufs=2, space="PSUM"))

    # ---- head DMA: first A pair leads, small bf16 operands slot behind ----
    at_tiles = {}
    an_tiles = {}

    def emit_load_pair(pr):
        an_tiles[pr] = an_stage.tile([P, 2 * N], BF16, tag="an", name=f"an_{pr}")
        nc.sync.dma_start(
            out=an_tiles[pr][:, :].rearrange("p (t n) -> p t n", t=2),
            in_=AN.rearrange("(t p) n -> p t n", p=P)[:, 2 * pr : 2 * pr + 2, :],
        )
        at_tiles[pr] = at_stage.tile([P, 2 * N], BF16, tag="at", name=f"at_{pr}")
        nc.sync.dma_start(
            out=at_tiles[pr][:, :].rearrange("p (t n) -> p t n", t=2),
            in_=AT.rearrange("(t p) n -> p t n", p=P)[:, 2 * pr : 2 * pr + 2, :],
        )

    emit_load_pair(0)
    # small constants ride the second HWDGE queue (ACT) alongside the A stream
    wt_sb = const.tile([P, FT * F], BF16, tag="wt")
    nc.scalar.dma_start(
        out=wt_sb[:, :].rearrange("p (c f) -> p c f", c=FT),
        in_=WTB.rearrange("(c p) f -> p c f", p=P),
    )
    b_bf = const.tile([1, F], BF16, tag="bbf")
    nc.scalar.dma_start(out=b_bf[:, :], in_=BIASB[:, :])
    xt_sb = const.tile([P, FT * N], BF16, tag="xt")
    nc.scalar.dma_start(
        out=xt_sb[:, :].rearrange("p (c n) -> p c n", c=FT),
        in_=XT.rearrange("(c p) n -> p c n", p=P),
    )
    for pr in range(1, PF_PAIRS):
        emit_load_pair(pr)

    # PE warmup: dummy matmuls with no data deps fill the idle head so the
    # HAM activity window un-throttles the PE clock before the real work
    from concourse.masks import make_identity

    ident = const.tile([P, P], BF16, tag="ident")
    make_identity(nc, ident[:, :])
    warm = psum_tr.tile([P, P], F32, tag="tr", name="warm")
    for _ in range(WARMUP_MMS):
        nc.tensor.matmul(warm[:, :], ident[:, :], ident[:, :], start=True, stop=True)

    ones_bf = const.tile([1, P], BF16, tag="ones")
    nc.vector.memset(ones_bf[:, :], 1.0)

    deg = const.tile([P, NT], F32, tag="deg")
    rec = const.tile([P, NT], F32, tag="rec")
    dinv = const.tile([P, NT], F32, tag="dinv")
    t_big = const.tile([P, NT * F], F32, tag="t")
    y_big = const.tile([P, NT * F], BF16, tag="y")

    # ---- mm1: t[j] = X @ W.T + b, bf16; two chains per PSUM bank so only
    # 8 wide drains are needed (2 emitted here, the rest inside the stream) ----
    tpp = {}
    for pj in range(NT // 2):
        tpp[pj] = psum_tr.tile([P, 2 * F], F32, tag="tr", name=f"tpp_{pj}")
        for jj in range(2):
            j = 2 * pj + jj
            reg = tpp[pj][:, jj * F : (jj + 1) * F]
            nc.tensor.matmul(
                reg, ones_bf[:, :], b_bf[:, :], start=(jj == 0), stop=False
            )
            for phi in range(FT):
                nc.tensor.matmul(
                    reg,
                    xt_sb[:, phi * N + j * P : phi * N + (j + 1) * P],
                    wt_sb[:, phi * F : (phi + 1) * F],
                    start=False,
                    stop=(jj == 1 and phi == FT - 1),
                )

    def emit_pair_drain(pj):
        dst = t_big[:, 2 * pj * F : (2 * pj + 2) * F]
        if pj % 2 == 0:
            nc.scalar.copy(dst, tpp[pj][:, :])
        else:
            nc.vector.tensor_copy(dst, tpp[pj][:, :])

    emit_pair_drain(0)
    emit_pair_drain(1)

    # ---- all 16 accumulation chains, 2 per bank ----
    acc_banks = [
        psum_acc.tile([P, 2 * F], F32, tag="acc", name=f"accbank_{b_}")
        for b_ in range(6)
    ]
    cbank_a = psum_tr.tile([P, 2 * F], F32, tag="tr", name="cbank_a")  # chains 12,13
    cbank_b = psum_tr.tile([P, 2 * F], F32, tag="tr", name="cbank_b")  # chains 14,15

    def acc_region(mu):
        half = (mu % 2) * F
        if mu < 12:
            return acc_banks[mu // 2][:, half : half + F]
        return (cbank_a if mu < 14 else cbank_b)[:, half : half + F]

    ostiles = {}

    def emit_drain(mu):
        bi = mu // STORE_BATCH
        if bi not in ostiles:
            ostiles[bi] = outstage.tile(
                [P, STORE_BATCH * F], F32, tag="os", name=f"os_{bi}"
            )
        j = mu % STORE_BATCH
        dst = ostiles[bi][:, j * F : (j + 1) * F]
        if mu % 2 == 0:
            # relu(d * psum) on ACT
            nc.scalar.activation(
                dst, acc_region(mu), RELU, scale=dinv[:, mu : mu + 1]
            )
        else:
            # same on DVE: (psum * d) max 0
            nc.vector.tensor_scalar(
                out=dst,
                in0=acc_region(mu),
                scalar1=dinv[:, mu : mu + 1],
                scalar2=0.0,
                op0=mybir.AluOpType.mult,
                op1=mybir.AluOpType.max,
            )
        if j == STORE_BATCH - 1:
            lo = bi * STORE_BATCH
            q = nc.sync if (bi % 2 == 0) else nc.gpsimd
            q.dma_start(
                out=OUT.rearrange("(m p) f -> p m f", p=P)[:, lo : lo + STORE_BATCH, :],
                in_=ostiles[bi][:, :].rearrange("p (m f) -> p m f", m=STORE_BATCH),
            )

    # ---- stream over the contraction index k ----
    def emit_products(k, at_pair_ap, mus):
        for mu in mus:
            nc.tensor.matmul(
                acc_region(mu),
                at_pair_ap[:, mu * P : (mu + 1) * P],
                y_big[:, k * F : (k + 1) * F],
                start=(k == 0 and mu % 2 == 0),
                stop=(k == NT - 1 and mu % 2 == 1),
            )

    at_aps = {}  # k -> AP of that tile within its pair (kept for lagged products)
    for pr in range(NP_):
        if pr + PF_PAIRS < NP_:
            emit_load_pair(pr + PF_PAIRS)
        at_pair = at_tiles.pop(pr)
        an_pair = an_tiles.pop(pr)
        for h in range(2):
            k = 2 * pr + h
            at_aps[k] = at_pair[:, h * N : (h + 1) * N]
            # degree row-sums, split across DVE (even k) and ACT (odd k)
            sc = scr.tile([P, N], BF16, tag="sc", name=f"sc_{k}")
            if k % 2 == 0:
                nc.vector.tensor_scalar(
                    out=sc[:, :],
                    in0=an_pair[:, h * N : (h + 1) * N],
                    scalar1=0.0,
                    scalar2=None,
                    op0=mybir.AluOpType.add,
                    op1=mybir.AluOpType.add,
                    accum_out=deg[:, k : k + 1],
                )
            else:
                nc.scalar.activation(
                    sc[:, :],
                    an_pair[:, h * N : (h + 1) * N],
                    COPY,
                    accum_out=deg[:, k : k + 1],
                )
            nc.vector.reciprocal(rec[:, k : k + 1], deg[:, k : k + 1])
            nc.scalar.sqrt(dinv[:, k : k + 1], rec[:, k : k + 1])
            nc.scalar.activation(
                y_big[:, k * F : (k + 1) * F],
                t_big[:, k * F : (k + 1) * F],
                COPY,
                scale=dinv[:, k : k + 1],
            )
            # remaining mm1 pair drains ride behind y early in the stream
            if k < 6:
                emit_pair_drain(k + 2)
            # products for resident chains now; tr-bank chains lag LAG_TR steps
            emit_products(k, at_aps[k], range(12))
            if k >= LAG_TR:
                emit_products(k - LAG_TR, at_aps[k - LAG_TR], range(12, NT))

    for k in range(NT - LAG_TR, NT):
        emit_products(k, at_aps[k], range(12, NT))

    # ---- tail: relu(d * acc) and batched stores ----
    for mu in range(NT):
        emit_drain(mu)


_cached_nc = None


def _build():
    nc = bacc.Bacc("TRN2", target_bir_lowering=False, debug=False)
    AT = nc.dram_tensor("at", [N, N], BF16, kind="ExternalInput").ap()
    AN = nc.dram_tensor("an", [N, N], BF16, kind="ExternalInput").ap()
    XT = nc.dram_tensor("xt", [F, N], BF16, kind="ExternalInput").ap()
    WTB = nc.dram_tensor("wtb", [F, F], BF16, kind="ExternalInput").ap()
    BIASB = nc.dram_tensor("biasb", [1, F], BF16, kind="ExternalInput").ap()
    OUT = nc.dram_tensor("out", [N, F], F32, kind="ExternalOutput").ap()
    with tile.TileContext(nc) as tc:
        with ExitStack() as ctx:
            _emit(ctx, tc, AT, AN, XT, WTB, BIASB, OUT)
    nc.compile()
    return nc


def get_nc():
    global _cached_nc
    if _cached_nc is None:
        _cached_nc = _build()
    return _cached_nc


def make_in_maps(node_features, adj_matrix, W, b):
    bf16 = ml_dtypes.bfloat16
    node_features = np.asarray(node_features, dtype=np.float32)
    adj_matrix = np.asarray(adj_matrix, dtype=np.float32)
    an = adj_matrix.astype(bf16)  # [B, N, N] natural
    at = np.ascontiguousarray(an.transpose(0, 2, 1))  # [B, N, N] transposed
    xt = np.ascontiguousarray(
        node_features.astype(bf16).transpose(0, 2, 1)
    )  # [B, F, N]
    wtb = np.ascontiguousarray(np.asarray(W, dtype=np.float32).T.astype(bf16))
    biasb = np.ascontiguousarray(
        np.asarray(b, dtype=np.float32).reshape(1, F).astype(bf16)
    )
    return [
        {
            "at": np.ascontiguousarray(at[c]),
            "an": np.ascontiguousarray(an[c]),
            "xt": xt[c],
            "wtb": wtb,
            "biasb": biasb,
        }
        for c in range(B)
    ]


def kernel(node_features, adj_matrix, W, b):
    nc = get_nc()
    in_maps = make_in_maps(node_features, adj_matrix, W, b)
    res = run_bass_kernel_spmd(nc, in_maps, core_ids=list(range(B)))
    return np.stack([r["out"] for r in res.results], axis=0)
